# revision 1
# baseline (speedup 1.0000x reference)
"""ConsensusAttention Trainium2 kernel.

Shapes (hardcoded): levels [B=8, N=1024, L=6, D=128] fp32.
Sharding: batch b across the 8 cores (data parallel); each core runs all
L=6 "heads" for its batch.

Math per (b, l):
  q = x, k = x / ||x||, sim[i, j] = (q_i . k_j) / sqrt(D)
  sim[i, i] = -0.0005 ; sim[i, j] = -inf where grid_dist(i, j) > 2
  out = softmax_j(sim) @ x

Key structure used by the kernel:
  * The 32x32-grid radius-2 mask only allows j - i in {0,+-1,+-2,+-31,
    +-32,+-33,+-64}; everything else is masked. So scores are banded
    (|i-j| <= 64) and each 128-row block of the score matrix only needs 3
    aligned 128-column blocks.
  * Scores are computed transposed, S'[j, i] = x_j . x_i, so that
    - the per-key normalization 1/(sqrt(D)*||x_j||) is a per-PARTITION
      scalar folded into the ACT Exp activation's vector `scale`,
    - softmax reduction over j and the attn @ V contraction over j are
      both matmuls with E[j, i] as the stationary operand.
  * Masking is done on the PE: an identity-weights matmul accumulates a
    precomputed bias tile (-60000 on masked entries) into the score PSUM.
  * V gets an appended ones-column so attn @ [V | 1] yields the softmax
    numerator and denominator in one PSUM tile.
  * The self-attention diagonal (constant exp(-0.0005) post-softmax-shift)
    rides as an extra c0*I matmul into each output accumulation.
  * All matmul operands are fp16 (1 col/cycle on the PE vs 4 for fp32, and
    exact fp32 PSUM accumulation).  fp16 inputs perturb the 128-term score
    reductions only ~3e-5; the attn@V products dominate the error at ~5e-4,
    well inside the fp32 gate.  Softmax needs no max-subtraction: scores
    are bounded by |q|/sqrt(D) ~= 1.
"""

from contextlib import ExitStack

import numpy as np

import concourse.bacc as bacc
import concourse.tile as tile
from concourse import mybir
from concourse.bass_utils import run_bass_kernel_spmd

B, N, L, D = 8, 1024, 6, 128
NB = N // 128  # 8 token blocks of 128
GRID = 32
RADIUS = 2.0
SELF_VAL = -0.0005
NEG = -1.0e30
F32 = mybir.dt.float32
F16 = mybir.dt.float16


def _chunk_base(jb: int) -> int:
    """First i-chunk index covered by j-block jb's 3-chunk (384 col) span."""
    return min(max(jb - 1, 0), NB - 3)


def _win(jb: int) -> int:
    """Start of the uniform 256-col score window for j-block jb.

    Covers the |i-j|<=64 band of the block; clipped shifts at the edges keep
    the width exactly 256 (extra columns are always-masked)."""
    return min(max(jb * 128 - 64, 0), N - 256)


def _build_constants():
    yy, xx = np.meshgrid(np.arange(GRID), np.arange(GRID), indexing="ij")
    coors = np.stack([yy.ravel(), xx.ravel()], axis=-1).astype(np.float32)
    dist = np.sqrt(((coors[:, None, :] - coors[None, :, :]) ** 2).sum(-1))
    bad = (dist > np.float32(RADIUS)) | np.eye(N, dtype=bool)  # [j, i] masked
    mb_full = np.where(bad, np.float32(NEG), np.float32(0.0)).astype(np.float32)

    # mb2[k] is the mask-bias for the PAIRED score bank of j-blocks
    # (2k, 2k+1): columns [0,256) mask block 2k's window, [256,512) block
    # 2k+1's window.  Partition p is row p of both blocks.  fp16: -60000
    # is plenty (exp(rs * -60000) == 0) and -1e30 would overflow.
    mb = np.empty((NB // 2, 128, 512), np.float16)
    for jb in range(NB):
        w0 = _win(jb)
        half = (jb % 2) * 256
        mb[jb // 2, :, half : half + 256] = np.where(
            bad[jb * 128 : (jb + 1) * 128, w0 : w0 + 256], -60000.0, 0.0
        ).astype(np.float16)
        # Every allowed (j, i) pair must fall inside the window.
        assert bad[jb * 128 : (jb + 1) * 128, :w0].all()
        assert bad[jb * 128 : (jb + 1) * 128, w0 + 256 :].all()

    ident = np.eye(128, dtype=np.float16)
    c0i = (np.exp(np.float32(SELF_VAL)) * np.eye(128)).astype(np.float16)
    return mb, np.stack([ident, c0i])


def _emit(tc: tile.TileContext, ctx: ExitStack, xh, mb, cns, out):
    nc = tc.nc
    const = ctx.enter_context(tc.tile_pool(name="const", bufs=1))
    xin = ctx.enter_context(tc.tile_pool(name="xin", bufs=1))
    xtp = ctx.enter_context(tc.tile_pool(name="xtp", bufs=2))
    small = ctx.enter_context(tc.tile_pool(name="small", bufs=4))
    scr = ctx.enter_context(tc.tile_pool(name="scr", bufs=2))
    epool = ctx.enter_context(tc.tile_pool(name="epool", bufs=3))
    opool = ctx.enter_context(tc.tile_pool(name="opool", bufs=3))
    tp = ctx.enter_context(tc.tile_pool(name="tp", bufs=2, space="PSUM"))
    sp = ctx.enter_context(tc.tile_pool(name="sp", bufs=3, space="PSUM"))
    op = ctx.enter_context(tc.tile_pool(name="op", bufs=3, space="PSUM"))

    xh_v = xh.rearrange("(b p) l d -> p b l d", p=128)
    out_v = out.rearrange("(b p) l d -> p b l d", p=128)

    # DMA order matters: head 0's data and the identity go first so compute
    # can start ~2us in; the big mask tile is only needed by the first mask
    # matmul and loads behind them.
    xh_all = xin.tile([128, NB, L, D + 1], F16, name="xh_all")
    nc.sync.dma_start(out=xh_all[:, :, 0, 0:D], in_=xh_v[:, :, 0, :])
    ident = const.tile([128, 128], F16, name="ident_sb")
    nc.scalar.dma_start(out=ident, in_=cns[0])
    c0ih = const.tile([128, 128], F16, name="c0ih")
    nc.scalar.dma_start(out=c0ih, in_=cns[1])
    mb_v = mb.rearrange("j p c -> p j c")
    mb_sb = const.tile([128, NB // 2, 512], F16, name="mb_sb")
    nc.scalar.dma_start(out=mb_sb[:, 0:2, :], in_=mb_v[:, 0:2, :])
    nc.sync.dma_start(out=mb_sb[:, 2:4, :], in_=mb_v[:, 2:4, :])
    for l in range(1, L):
        eng = nc.sync if l % 2 == 0 else nc.scalar
        eng.dma_start(out=xh_all[:, :, l, 0:D], in_=xh_v[:, :, l, :])
    nc.vector.memset(xh_all[:, :, :, D : D + 1], 1.0)

    # E tiles live in a fixed 384-wide aligned 3-chunk frame per j-block.
    # Only the 256-col score window inside the frame is ever computed; the
    # band-edge strips are zeroed ONCE here and never written again, so the
    # attn@V matmuls can read full aligned 128-col chunks.  Tiles rotate
    # manually per frame class (jb==0 / interior / jb==7) so each slot's
    # zero strips are stable across reuse.
    e_edge0 = const.tile([128, 384], F16, tag="e_edge0", name="e_edge0")
    nc.vector.memset(e_edge0[:, 256:384], 0.0)
    e_edge7 = const.tile([128, 384], F16, tag="e_edge7", name="e_edge7")
    nc.vector.memset(e_edge7[:, 0:128], 0.0)
    e_mid = []
    for k in range(4):
        t = const.tile([128, 384], F16, tag=f"e_mid{k}", name=f"e_mid{k}")
        nc.vector.memset(t[:, 0:64], 0.0)
        nc.vector.memset(t[:, 320:384], 0.0)
        e_mid.append(t)
    mid_uses = 0

    # norm2[p, l, b] = sum_d x^2; squares on GPSIMD (otherwise idle).
    # fp16 x only perturbs norms/scores ~3e-5: the per-element rounding
    # averages out across the 128-term reductions (fp32 accumulate).
    # rs = 1/sqrt(D * norm2) via exp(-0.5 * ln(.)): Ln+Exp are batched in two
    # groups (heads 0-1, then 2-5) — few ACT table switches, but the first
    # pairs don't stall behind all six square passes.  Sqrt lives in a table
    # set without exp and would force a reload per head.
    norm2 = small.tile([128, L, NB], F32, name="norm2_all")
    lnn = small.tile([128, L, NB], F32, name="lnn_all")
    rs_all = small.tile([128, L, NB], F32, name="rs_all")

    def emit_norms(l_lo, l_hi):
        for l in range(l_lo, l_hi):
            # fp16 squares run in the DVE's 2x packed mode (~0.6us vs 2.2us
            # on GPSIMD); split across both engines so a batch completes in
            # about one op-latency.
            sq = scr.tile([128, NB, D], F16, tag="sq", name=f"sq_{l}")
            eng = nc.gpsimd if l % 2 == 0 and l >= 2 else nc.vector
            eng.tensor_mul(sq, xh_all[:, :, l, 0:D], xh_all[:, :, l, 0:D])
            nc.vector.reduce_sum(
                norm2[:, l, :], sq, axis=mybir.AxisListType.X
            )
        nc.scalar.activation(
            lnn[:, l_lo:l_hi, :],
            norm2[:, l_lo:l_hi, :],
            mybir.ActivationFunctionType.Ln,
            scale=float(D),
        )
        nc.scalar.activation(
            rs_all[:, l_lo:l_hi, :],
            lnn[:, l_lo:l_hi, :],
            mybir.ActivationFunctionType.Exp,
            scale=-0.5,
        )

    def emit_transposes(l):
        # XT[d, token] via fp16 PE transposes — all 8 blocks fit one PSUM
        # bank ([128, 1024] fp16 = 2KB/partition) -> a single copy to SBUF.
        xt = xtp.tile([128, N], F16, tag="xt", name=f"xt_{l}")
        pt = tp.tile([128, N], F16, tag="pt", name=f"pt_{l}")
        for b in range(NB):
            nc.tensor.matmul(
                pt[:, b * 128 : (b + 1) * 128],
                lhsT=xh_all[:, b, l, 0:D],
                rhs=ident,
                is_transpose=True,
                start=(b == 0),
                stop=(b == NB - 1),
            )
        nc.vector.tensor_copy(out=xt, in_=pt)
        return xt

    # Head 0's transposes go first: their DVE copy must not queue behind
    # the norm chain, so PE starts score matmuls ~2us into the kernel.
    xt_0 = emit_transposes(0)
    emit_norms(0, 2)

    for l in range(L):
        if l == 2:
            emit_norms(2, L)
        rs = rs_all[:, l, :]
        xt = xt_0 if l == 0 else emit_transposes(l)

        stage = opool.tile([128, NB, D], F32, tag="stage", name=f"stage_{l}")
        o_ps = {}
        for jb in range(NB):
            cb = _chunk_base(jb)
            w0 = _win(jb)
            fo = w0 - cb * 128  # window offset inside the 384 frame
            h = jb % 2  # which half of the paired mask layout to use
            # S'[j, i-window] = x_j . x_i  + mask bias (identity matmul),
            # all fp16 (1 col/cycle on the PE; fp32 accumulation in PSUM).
            s_ps = sp.tile([128, 256], F32, tag="s", name=f"s_{l}_{jb}")
            nc.tensor.matmul(
                s_ps,
                lhsT=xt[:, jb * 128 : (jb + 1) * 128],
                rhs=xt[:, w0 : w0 + 256],
                start=True,
                stop=False,
            )
            nc.tensor.matmul(
                s_ps,
                lhsT=ident,
                rhs=mb_sb[:, jb // 2, h * 256 : (h + 1) * 256],
                start=False,
                stop=True,
            )
            # E = exp(rs[j] * S') with rs as per-partition ACT scale,
            # written into the window range of the fixed 384-wide frame.
            if jb == 0:
                e = e_edge0
            elif jb == NB - 1:
                e = e_edge7
            else:
                e = e_mid[mid_uses % len(e_mid)]
                mid_uses += 1
            nc.scalar.activation(
                e[:, fo : fo + 256],
                s_ps,
                mybir.ActivationFunctionType.Exp,
                scale=rs[:, jb : jb + 1],
            )
            # attn @ [V|1]: full aligned chunks (zero strips add nothing).
            # The self-attention diagonal contribution (weight exp(-0.0005)
            # for token i onto itself) rides as an extra c0*I matmul into
            # each output accumulation instead of patching E.
            for c in range(3):
                ib = cb + c
                if abs(ib - jb) > 1:
                    continue
                first = jb == max(ib - 1, 0)
                last = jb == min(ib + 1, NB - 1)
                if first:
                    o_ps[ib] = op.tile(
                        [128, D + 1], F32, tag="o", name=f"o_{l}_{ib}"
                    )
                nc.tensor.matmul(
                    o_ps[ib],
                    lhsT=e[:, c * 128 : (c + 1) * 128],
                    rhs=xh_all[:, jb, l, :],
                    start=first,
                    stop=last,
                )
                if first:
                    # first != last always (every block has >= 2 j-block
                    # contributors), so the group is still open here.
                    nc.tensor.matmul(
                        o_ps[ib],
                        lhsT=c0ih,
                        rhs=xh_all[:, ib, l, :],
                        start=False,
                        stop=False,
                    )
                if last:
                    ot = o_ps.pop(ib)
                    rcp = small.tile(
                        [128, 1], F32, tag="rcp", name=f"rcp_{l}_{ib}"
                    )
                    nc.vector.reciprocal(rcp, ot[:, D : D + 1])
                    nc.any.tensor_scalar_mul(stage[:, ib, :], ot[:, 0:D], rcp)
        assert not o_ps
        # One output DMA per head, alternating HWDGE queues.
        eng = nc.scalar if l % 2 == 0 else nc.sync
        eng.dma_start(out=out_v[:, :, l, :], in_=stage)


def build_nc():
    nc = bacc.Bacc("TRN2", target_bir_lowering=False, debug=False, num_devices=B)
    xh = nc.dram_tensor("xh", [N, L, D], F16, kind="ExternalInput").ap()
    mb = nc.dram_tensor("mb", [NB // 2, 128, 512], F16, kind="ExternalInput").ap()
    cns = nc.dram_tensor("cns", [2, 128, 128], F16, kind="ExternalInput").ap()
    out = nc.dram_tensor("out", [N, L, D], F32, kind="ExternalOutput").ap()
    with tile.TileContext(nc) as tc:
        with ExitStack() as ctx:
            _emit(tc, ctx, xh, mb, cns, out)
    nc.compile()
    return nc


_NC = None


def _get_nc():
    global _NC
    if _NC is None:
        _NC = build_nc()
    return _NC


def run_spmd(levels: np.ndarray, trace: bool = False):
    """Run on the 8 NeuronCores; returns (out [B,N,L,D], exec_time_ns|None)."""
    levels = np.ascontiguousarray(levels, dtype=np.float32)
    assert levels.shape == (B, N, L, D), levels.shape
    mb, cns = _build_constants()
    nc = _get_nc()
    xh = levels.astype(np.float16)
    in_maps = [{"xh": xh[b], "mb": mb, "cns": cns} for b in range(B)]
    res = run_bass_kernel_spmd(
        nc, in_maps, core_ids=list(range(B)), trace=trace
    )
    out = np.stack([res.results[b]["out"] for b in range(B)]).astype(np.float32)
    return out, res.exec_time_ns


def kernel(levels: np.ndarray) -> np.ndarray:
    out, _ = run_spmd(levels, trace=False)
    return out



# revision 8
# speedup vs baseline: 1.0072x; 1.0072x over previous
"""ConsensusAttention Trainium2 kernel (v2).

Shapes (hardcoded): levels [B=8, N=1024, L=6, D=128] fp32.
Sharding: batch b across the 8 cores (data parallel); each core runs all
L=6 "heads" for its batch.

Math per (b, l):
  q = x, k = x / ||x||, sim[i, j] = (q_i . k_j) / sqrt(D)
  sim[i, i] = -0.0005 ; sim[i, j] = -inf where grid_dist(i, j) > 2
  out = softmax_j(sim) @ x

Structure (see v1 docstring for the banded-mask derivation):
  * 32x32-grid radius-2 mask => scores banded (|i-j| <= 64); each 128-row
    j-block of the transposed score matrix S'[j, i] only needs a 256-col
    i-window.  E tiles live in per-jb 384-wide frames [z64|win256|z64] so
    attn@V reads aligned 128-col chunks (zero strips cover the overhang).
  * Scores for a j-block PAIR share one 2KB PSUM bank; ONE identity-lhsT
    matmul per pair adds the (-60000 masked / 0) bias for both halves.
  * rs[j] = 1/(sqrt(D)*||x_j||) is computed entirely on the DVE via the
    Quake rsqrt bit-trick + 2 Newton iterations (fp32) - no Ln activation,
    so the ACT engine only ever loads the Exp table (once).
  * attn@V: lhsT = E chunks, rhs = [V | 1] (ones column -> denominator in
    the same PSUM tile).  Self-attention diagonal rides as a c0*I matmul.
  * o-accumulators packed 3 per PSUM bank; normalize = one tensor_scalar
    divide per block (numerator / denominator, per-partition scalar).
  * All matmul operands fp16 (1 col/cycle); fp32 PSUM accumulation.
  * All DMAs issue from the SP queue (HWDGE) - keeps ACT/DVE sequencers
    free; head-0 input load is split in halves to cut the cold start.
"""

from contextlib import ExitStack

import numpy as np

import concourse.bacc as bacc
import concourse.tile as tile
from concourse import mybir
from concourse.alu_op_type import AluOpType
from concourse.bass_utils import run_bass_kernel_spmd

B, N, L, D = 8, 1024, 6, 128
NB = N // 128  # 8 token blocks of 128
NP = NB // 2  # 4 j-block pairs
GRID = 32
RADIUS = 2.0
SELF_VAL = -0.0005
F32 = mybir.dt.float32
F16 = mybir.dt.float16
I32 = mybir.dt.int32
RSQRT_MAGIC = 0x5F3759DF
INV_SQRT_D = float(D) ** -0.5


def _win(jb: int) -> int:
    """Start of the 256-col score window for j-block jb (covers the
    |i-j|<=64 band; clipped at the edges)."""
    return min(max(jb * 128 - 64, 0), N - 256)


def _contrib(jb: int):
    """Output blocks that j-block jb contributes to."""
    return [ib for ib in (jb - 1, jb, jb + 1) if 0 <= ib < NB]


def _build_constants():
    yy, xx = np.meshgrid(np.arange(GRID), np.arange(GRID), indexing="ij")
    coors = np.stack([yy.ravel(), xx.ravel()], axis=-1).astype(np.float32)
    dist = np.sqrt(((coors[:, None, :] - coors[None, :, :]) ** 2).sum(-1))
    bad = (dist > np.float32(RADIUS)) | np.eye(N, dtype=bool)  # [j, i] masked

    # mb[p] = mask bias for the paired score bank of j-blocks (2p, 2p+1):
    # cols [0,256) mask block 2p's window, [256,512) block 2p+1's.
    mb = np.empty((NP, 128, 512), np.float16)
    for jb in range(NB):
        w0 = _win(jb)
        half = (jb % 2) * 256
        mb[jb // 2, :, half : half + 256] = np.where(
            bad[jb * 128 : (jb + 1) * 128, w0 : w0 + 256], -60000.0, 0.0
        ).astype(np.float16)
        # Every allowed (j, i) pair must fall inside the window, and every
        # aligned-chunk overhang outside the window must be fully masked
        # (those E positions are the frame's zero strips).
        assert bad[jb * 128 : (jb + 1) * 128, :w0].all()
        assert bad[jb * 128 : (jb + 1) * 128, w0 + 256 :].all()
        for ib in _contrib(jb):
            off = ib * 128 - w0  # chunk start relative to window
            assert -64 <= off <= 192, (jb, ib, off)

    ident = np.eye(128, dtype=np.float16)
    c0i = (np.exp(np.float32(SELF_VAL)) * np.eye(128)).astype(np.float16)
    return mb, np.stack([ident, c0i])


def _emit(tc: tile.TileContext, ctx: ExitStack, xh, mb, cns, out):
    nc = tc.nc
    const = ctx.enter_context(tc.tile_pool(name="const", bufs=1))
    xin = ctx.enter_context(tc.tile_pool(name="xin", bufs=1))
    xtp = ctx.enter_context(tc.tile_pool(name="xtp", bufs=2))
    small = ctx.enter_context(tc.tile_pool(name="small", bufs=4))
    scr = ctx.enter_context(tc.tile_pool(name="scr", bufs=2))
    stg = ctx.enter_context(tc.tile_pool(name="stg", bufs=2))
    tp = ctx.enter_context(tc.tile_pool(name="tp", bufs=2, space="PSUM"))
    sp = ctx.enter_context(tc.tile_pool(name="sp", bufs=3, space="PSUM"))
    op = ctx.enter_context(tc.tile_pool(name="op", bufs=3, space="PSUM"))

    xh_v = xh.rearrange("(b p) l d -> p b l d", p=128)
    out_v = out.rearrange("(b p) l d -> p b l d", p=128)
    mb_v = mb.rearrange("j p c -> p j c")

    # --- input DMAs, all on the SP queue (HWDGE; keeps ACT/DVE seq free).
    # Order: constants -> head-0 halves -> mask -> heads 1..5.
    ident = const.tile([128, 128], F16, name="ident_sb")
    c0ih = const.tile([128, 128], F16, name="c0ih")
    nc.sync.dma_start(out=ident, in_=cns[0])
    nc.sync.dma_start(out=c0ih, in_=cns[1])
    xh_all = xin.tile([128, NB, L, D + 1], F16, name="xh_all")
    nc.sync.dma_start(out=xh_all[:, 0:4, 0, 0:D], in_=xh_v[:, 0:4, 0, :])
    nc.sync.dma_start(out=xh_all[:, 4:8, 0, 0:D], in_=xh_v[:, 4:8, 0, :])
    mb_sb = const.tile([128, NP, 512], F16, name="mb_sb")
    nc.sync.dma_start(out=mb_sb, in_=mb_v)
    for l in range(1, L):
        nc.sync.dma_start(out=xh_all[:, :, l, 0:D], in_=xh_v[:, :, l, :])
    nc.vector.memset(xh_all[:, :, :, D : D + 1], 1.0)

    # E frames: [128, 2, 384] per j-block pair; each subframe is
    # [z64 | win 256 | z64].  Zero strips memset ONCE (gpsimd - idle engine)
    # and never rewritten; tiles rotate manually so strips stay valid.
    e_tiles = []
    for k in range(3):
        t = const.tile([128, 2, 384], F16, tag=f"e{k}", name=f"e{k}")
        nc.gpsimd.memset(t[:, :, 0:64], 0.0)
        nc.gpsimd.memset(t[:, :, 320:384], 0.0)
        e_tiles.append(t)

    norm2 = small.tile([128, L, NB], F32, name="norm2")
    rs = small.tile([128, L, NB], F32, name="rs")

    def emit_norms(l, blo, bhi):
        # sum_d x^2 per token, fp16 squares in DVE 2x packed mode.
        sq = scr.tile([128, NB, D], F16, tag="sq", name=f"sq_{l}_{blo}")
        nc.vector.tensor_mul(
            sq[:, blo:bhi],
            xh_all[:, blo:bhi, l, 0:D],
            xh_all[:, blo:bhi, l, 0:D],
        )
        nc.vector.reduce_sum(
            norm2[:, l, blo:bhi], sq[:, blo:bhi], axis=mybir.AxisListType.X
        )

    def emit_rs(llo, lhi):
        # rs = (1/sqrt(D)) * rsqrt(norm2), Quake seed + 2 Newton steps.
        # All fp32 on the DVE; rel err ~4e-6 - well inside the fp16 noise.
        g = lhi - llo
        a = norm2[:, llo:lhi, :]
        y = small.tile([128, g, NB], F32, tag="nr_y", name=f"y_{llo}")
        t = small.tile([128, g, NB], F32, tag="nr_t", name=f"t_{llo}")
        # seed: y = bitcast(MAGIC - (bits(a) >> 1)) == bitcast(-(bits>>1)*1 + MAGIC)
        nc.vector.tensor_scalar(
            y.bitcast(I32),
            a.bitcast(I32),
            1,
            None,
            op0=AluOpType.logical_shift_right,
        )
        nc.vector.tensor_scalar(
            y.bitcast(I32), y.bitcast(I32), -1, RSQRT_MAGIC,
            op0=AluOpType.mult, op1=AluOpType.add,
        )
        for last in (False, True):
            nc.vector.tensor_mul(t, y, y)  # y^2
            nc.vector.tensor_mul(t, t, a)  # a*y^2
            # w = 1.5 - 0.5*a*y^2  (fold 1/sqrt(D) into the last step)
            c = INV_SQRT_D if last else 1.0
            nc.vector.tensor_scalar(
                t, t, -0.5 * c, 1.5 * c, op0=AluOpType.mult, op1=AluOpType.add
            )
            nc.vector.tensor_mul(rs[:, llo:lhi, :] if last else y, y, t)

    def emit_transposes(l, half, pt, xt):
        # XT[d, token] via fp16 PE transposes; one 512-col half at a time so
        # head 0 can start on its first DMA half.
        for b in range(4 * half, 4 * half + 4):
            nc.tensor.matmul(
                pt[:, b * 128 : (b + 1) * 128],
                lhsT=xh_all[:, b, l, 0:D],
                rhs=ident,
                is_transpose=True,
                start=(b % 4 == 0),
                stop=(b % 4 == 3),
            )
        nc.vector.tensor_copy(
            out=xt[:, half * 512 : (half + 1) * 512],
            in_=pt[:, half * 512 : (half + 1) * 512],
        )

    def new_xt(l):
        pt = tp.tile([128, N], F16, tag="pt", name=f"pt_{l}")
        xt = xtp.tile([128, N], F16, tag="xt", name=f"xt_{l}")
        return pt, xt

    # Head 0 front matter: transposes + norms as the two DMA halves land.
    pt0, xt0 = new_xt(0)
    emit_transposes(0, 0, pt0, xt0)
    emit_norms(0, 0, 4)
    emit_transposes(0, 1, pt0, xt0)
    emit_norms(0, 4, 8)
    emit_rs(0, 1)

    xt_cur = xt0
    ei = 0  # rotating E-frame index

    for l in range(L):
        xt = xt_cur

        def scores(p, e):
            # Two 256-col score matmuls into one PSUM pair bank + ONE
            # mask-bias matmul for the whole pair, then per-half Exp with
            # rs as the per-partition ACT scale.
            s_ps = sp.tile([128, 2, 256], F32, tag="s", name=f"s_{l}_{p}")
            for h in range(2):
                jb = 2 * p + h
                w0 = _win(jb)
                nc.tensor.matmul(
                    s_ps[:, h, :],
                    lhsT=xt[:, jb * 128 : (jb + 1) * 128],
                    rhs=xt[:, w0 : w0 + 256],
                    start=(h == 0),
                    stop=False,
                    skip_group_check=True,
                )
            nc.tensor.matmul(
                s_ps[:, :, :],
                lhsT=ident,
                rhs=mb_sb[:, p, :],
                start=False,
                stop=True,
                skip_group_check=True,
            )
            for h in range(2):
                jb = 2 * p + h
                nc.scalar.activation(
                    e[:, h, 64:320],
                    s_ps[:, h, :],
                    mybir.ActivationFunctionType.Exp,
                    scale=rs[:, l, jb : jb + 1],
                )

        o_banks = {}
        closed = {}

        def attnv(jb, e):
            h = jb % 2
            w0 = _win(jb)
            for ib in _contrib(jb):
                off = 64 + ib * 128 - w0  # chunk start in the 384 frame
                k, slot = divmod(ib, 3)
                first = jb == max(ib - 1, 0)
                last = jb == min(ib + 1, NB - 1)
                if first:
                    bank_start = k not in o_banks
                    if bank_start:
                        o_banks[k] = op.tile(
                            [128, 3, D + 1], F32, tag="o", name=f"o_{l}_{k}"
                        )
                        closed[k] = 0
                    nc.tensor.matmul(
                        o_banks[k][:, slot, :],
                        lhsT=c0ih,
                        rhs=xh_all[:, ib, l, :],
                        start=bank_start,
                        stop=False,
                        skip_group_check=True,
                    )
                nc.tensor.matmul(
                    o_banks[k][:, slot, :],
                    lhsT=e[:, h, off : off + 128],
                    rhs=xh_all[:, jb, l, :],
                    start=False,
                    stop=last,
                    skip_group_check=True,
                )
                if last:
                    closed[k] += 1
                    nblk = 2 if k == 2 else 3
                    if closed[k] == nblk:
                        ob = o_banks.pop(k)
                        rcp = small.tile(
                            [128, nblk], F32, tag="rcp", name=f"rcp_{l}_{k}"
                        )
                        nc.vector.reciprocal(rcp, ob[:, 0:nblk, D])
                        for s2 in range(nblk):
                            ib2 = k * 3 + s2
                            nc.vector.tensor_scalar_mul(
                                stage[:, ib2, :],
                                ob[:, s2, 0:D],
                                rcp[:, s2 : s2 + 1],
                            )

        stage = stg.tile([128, NB, D], F32, tag="stage", name=f"stage_{l}")
        e_p = []
        for p in range(NP):
            e = e_tiles[ei % 3]
            ei += 1
            e_p.append(e)
            scores(p, e)
            if p == 1:
                # Hide exp latency: head l+1's transposes + next heads' norms
                # (rs for head l+1 must be ready before head l+1's first exp).
                if l + 1 < L:
                    pt_n, xt_n = new_xt(l + 1)
                    emit_transposes(l + 1, 0, pt_n, xt_n)
                    emit_transposes(l + 1, 1, pt_n, xt_n)
                    xt_cur = xt_n
                if l == 0:
                    emit_norms(1, 0, 8)
                    emit_rs(1, 2)
                elif l == 1:
                    emit_norms(2, 0, 8)
                    emit_norms(3, 0, 8)
                    emit_rs(2, 4)
                elif l == 2:
                    emit_norms(4, 0, 8)
                    emit_norms(5, 0, 8)
                    emit_rs(4, 6)
                attnv(0, e_p[0])
                attnv(1, e_p[0])
            elif p == 2:
                attnv(2, e_p[1])
                attnv(3, e_p[1])
            elif p == 3:
                attnv(4, e_p[2])
                attnv(5, e_p[2])
        attnv(6, e_p[3])
        attnv(7, e_p[3])
        assert not o_banks
        nc.sync.dma_start(out=out_v[:, :, l, :], in_=stage)


def build_nc():
    nc = bacc.Bacc("TRN2", target_bir_lowering=False, debug=False, num_devices=B)
    xh = nc.dram_tensor("xh", [N, L, D], F16, kind="ExternalInput").ap()
    mb = nc.dram_tensor("mb", [NP, 128, 512], F16, kind="ExternalInput").ap()
    cns = nc.dram_tensor("cns", [2, 128, 128], F16, kind="ExternalInput").ap()
    out = nc.dram_tensor("out", [N, L, D], F32, kind="ExternalOutput").ap()
    with tile.TileContext(nc) as tc:
        with ExitStack() as ctx:
            _emit(tc, ctx, xh, mb, cns, out)
    nc.compile()
    return nc


_NC = None


def _get_nc():
    global _NC
    if _NC is None:
        _NC = build_nc()
    return _NC


def run_spmd(levels: np.ndarray, trace: bool = False):
    """Run on the 8 NeuronCores; returns (out [B,N,L,D], exec_time_ns|None)."""
    levels = np.ascontiguousarray(levels, dtype=np.float32)
    assert levels.shape == (B, N, L, D), levels.shape
    mb, cns = _build_constants()
    nc = _get_nc()
    xh = levels.astype(np.float16)
    in_maps = [{"xh": xh[b], "mb": mb, "cns": cns} for b in range(B)]
    res = run_bass_kernel_spmd(
        nc, in_maps, core_ids=list(range(B)), trace=trace
    )
    out = np.stack([res.results[b]["out"] for b in range(B)]).astype(np.float32)
    return out, res.exec_time_ns


def kernel(levels: np.ndarray) -> np.ndarray:
    out, _ = run_spmd(levels, trace=False)
    return out
